# revision 30
# baseline (speedup 1.0000x reference)
"""Trainium2 Bass kernel for nn_LogicLayer (differentiable logic-gate layer).

Math:
    a = x[:, idx_a]; b = x[:, idx_b]                  # gather columns
    c = softmax(weights) @ T                          # [O, 4] truth-table coeffs
    out = c0*(1-a)(1-b) + c1*(1-a)b + c2*a(1-b) + c3*ab
        = k0 + ka*a + kb*b + kab*a*b
  with k0 = c0, ka = c2-c0, kb = c1-c0, kab = c0-c1-c2+c3.

Device strategy (8 cores, out_dim sharded, 2048 gates/core):
  - Host centers x (t = x - 0.5) and pre-transposes to tT [in_dim, B] so a
    gate's input column is a contiguous 8KB row; stored fp16 (MODE 'fp16')
    or int16 fixed-point (MODE 'i16'). Folded coefficients:
      out = K0 + KA*ta + KB*tb + KAB*ta*tb
      K0 = k0+ka/2+kb/2+kab/4, KA = al*(ka+kab/2),
      KB = al*(kb+kab/2),      KAB = al*al*kab,  al = 1 (fp16) or 1/65536.
  - Per 128-gate block: indirect-DMA gather the 128 a-columns and b-columns
    of tT into SBUF as [128 gates, 4096 batch] fp16 tiles; coefficients are
    per-partition scalars: u = KAB*ta+KB (ScalarE act), v = KA*ta+K0 (DVE
    tensor_scalar 4x), p = u*tb (DVE 2x), s = p+v (DVE 2x); PE is_transpose
    flips 128x128 blocks of s to [batch, gate] fp16 PSUM tiles; ScalarE/DVE
    copy PSUM into a [128, 32, 512] group buffer covering 4 gate blocks, so
    one 4MB output DMA per group writes contiguous 1KB runs (big write
    bursts minimize HBM read/write turnaround against the gather stream).
    Output DRAM tensor is fp16; host upcasts to f32 (adds rel err ~2^-11,
    far under the 2e-2 gate; measured max rel err 3.6e-3).

  Measured on 8 axon trn2 cores: 255.7us (staged int16/f32 baseline) ->
  ~162us. Per-core DMA floor is ~141-152us (33.5MB gathers + 16.8MB out at
  330-358 GB/s); engines (~100us) hide under it. fp16-PSUM accumulation and
  DMA-CCE-mult folding were tried and are unsupported (see git-less notes:
  PSUM packs 2 fp16 per f32 slot; CCE mult fails NEFF compile).
"""

import contextlib

import numpy as np

import concourse.bass as bass
import concourse.tile as tile
from concourse import bacc, mybir
from concourse.bass_utils import run_bass_kernel_spmd
from concourse.masks import make_identity

B = 4096          # batch
IN_DIM = 4096     # input features
O = 16384         # gates (out_dim)
NCORES = 8
OSH = O // NCORES  # 2048 gates per core
P = 128
GBLOCKS = OSH // P  # 16 gate blocks per core
GRP = 4             # gate blocks per output DMA group (512 gates, 1KB runs)

# 'fp16': fp16 gathers + fp16 u/p on DVE (2-4x modes), f32 v on ScalarE.
# 'i16' : int16 gathers + f32 compute (safer precision), fp16 out.
MODE = "fp16"
PSUM_ACC = False  # fp16 PSUM accumulate is broken (2 elems packed per f32 slot)
OUT_MODE = "one"  # output DMA batching: "q4" 4x1MB | "pair" 2x2MB | "one" 1x4MB
REGMM = False     # transpose p/v via regular matmuls (rhs=identity) accumulating
                  # in f32 PSUM — drops the s-pass without fp16-PSUM issues
COPY_PAT = "sce2of3"  # PSUM->SBUF copy split: even | sce2of3 | dve2of3 | sce3of4
INPLACE_S = False # write s over v's tile (frees one tmp tag for more gather bufs)
INPLACE_P = False # write p over u's tile
GATHER_MULT = False  # fold p = u*b into the b-gather via DMA CCE mult
GATH_BUFS = 4     # gather pool depth (fp16 mode)
OT_BUFS = 2       # output group buffer depth
S_ENG = "dve"     # engine for the s = p+v pass: dve | gpsimd
SPLITC = 1        # split u/v/p/s into SPLITC half-batch tiles for finer overlap
MERGE_GATHER = False  # one indirect DMA per block gathering both a and b
                      # columns ([128, 2] offset AP, 256 descriptors/instr)
OUT_SCRAMBLE = False  # write group buffers verbatim (contiguous 4MB) and let
                      # the host unscramble [grp,p,j,g] -> [B, OSH]
PSUM_BUFS = 4
COPY2Q = False    # one PSUM tile + copy per 2 batch quarters (2 banks, FD 2048)

_PROGRAMS = {}


def _build_program(reps=1, mode=MODE):
    f32 = mybir.dt.float32
    f16 = mybir.dt.float16
    i32 = mybir.dt.int32
    xdt = f16 if mode == "fp16" else mybir.dt.int16

    nc = bacc.Bacc(None)
    xt_d = nc.declare_dram_parameter("xt", [IN_DIM, B], xdt, isOutput=False)
    ia_d = nc.declare_dram_parameter("idxa", [P, GBLOCKS], i32, isOutput=False)
    ib_d = nc.declare_dram_parameter("idxb", [P, GBLOCKS], i32, isOutput=False)
    if MERGE_GATHER:
        iab_d = nc.declare_dram_parameter(
            "idxab", [P, 2 * GBLOCKS], i32, isOutput=False
        )
    coef_d = nc.declare_dram_parameter("coef", [P, GBLOCKS * 4], f32, isOutput=False)
    if OUT_SCRAMBLE:
        assert OUT_MODE == "one"
        ngrp = GBLOCKS // GRP
        out_d = nc.declare_dram_parameter(
            "out", [ngrp * P, 32 * GRP * P], f16, isOutput=True
        )
        out_v = None
    else:
        out_d = nc.declare_dram_parameter("out", [B, OSH], f16, isOutput=True)
        # view: [batch_block j, 128 p, gate] -> [p, j, gate] for strided stores
        out_v = out_d[:].rearrange("(j p) g -> p j g", p=P)

    with tile.TileContext(nc) as tc:
        with (
            tc.tile_pool(name="const", bufs=1) as const_pool,
            tc.tile_pool(name="gath", bufs=GATH_BUFS if mode == "fp16" else 2) as gath_pool,
            tc.tile_pool(name="tmp", bufs=2) as tmp_pool,
            tc.tile_pool(name="ot", bufs=OT_BUFS) as ot_pool,
            tc.tile_pool(name="psum", bufs=PSUM_BUFS, space="PSUM") as psum_pool,
        ):
            ident_p = const_pool.tile([P, P], f16 if mode == "fp16" else f32)
            make_identity(nc, ident_p[:])
            if mode == "fp16":
                ident_v = const_pool.tile([P, P], f32)
                make_identity(nc, ident_v[:])
            else:
                ident_v = ident_p

            idxa_t = const_pool.tile([P, GBLOCKS], i32)
            nc.sync.dma_start(out=idxa_t[:], in_=ia_d[:])
            idxb_t = const_pool.tile([P, GBLOCKS], i32)
            nc.sync.dma_start(out=idxb_t[:], in_=ib_d[:])
            if MERGE_GATHER:
                idxab_t = const_pool.tile([P, 2 * GBLOCKS], i32)
                nc.sync.dma_start(out=idxab_t[:], in_=iab_d[:])
            coef_t = const_pool.tile([P, GBLOCKS * 4], f32)
            nc.sync.dma_start(out=coef_t[:], in_=coef_d[:])

            loop_cm = (
                tc.For_i(0, reps, 1) if reps > 1 else contextlib.nullcontext()
            )
            with loop_cm:
                n_ot = {"q4": 4, "pair": 2, "one": 1}[OUT_MODE]
                jspan = 32 // n_ot
                for grp in range(GBLOCKS // GRP):
                    qts = [
                        ot_pool.tile(
                            [P, jspan, GRP * P], f16, tag=f"q{q}", name=f"qt{q}"
                        )
                        for q in range(n_ot)
                    ]
                    for g4 in range(GRP):
                        gb = grp * GRP + g4
                        K0 = coef_t[:, 4 * gb : 4 * gb + 1]
                        KA = coef_t[:, 4 * gb + 1 : 4 * gb + 2]
                        KB = coef_t[:, 4 * gb + 2 : 4 * gb + 3]
                        KAB = coef_t[:, 4 * gb + 3 : 4 * gb + 4]

                        if mode == "fp16" and MERGE_GATHER:
                            ab_t = gath_pool.tile([P, 2, B], xdt, tag="ab")
                            nc.gpsimd.indirect_dma_start(
                                out=ab_t[:],
                                out_offset=None,
                                in_=xt_d[:],
                                in_offset=bass.IndirectOffsetOnAxis(
                                    ap=idxab_t[:, 2 * gb : 2 * gb + 2], axis=0
                                ),
                            )
                            ab_flat = ab_t[:].rearrange("p c b -> p (c b)")
                            a_t = ab_flat[:, 0:B]
                            b_t = ab_flat[:, B : 2 * B]
                        else:
                            a_t = gath_pool.tile([P, B], xdt, tag="a")
                            nc.gpsimd.indirect_dma_start(
                                out=a_t[:],
                                out_offset=None,
                                in_=xt_d[:],
                                in_offset=bass.IndirectOffsetOnAxis(
                                    ap=idxa_t[:, gb : gb + 1], axis=0
                                ),
                            )
                        if mode == "fp16" and MERGE_GATHER:
                            pass
                        elif not (mode == "fp16" and GATHER_MULT):
                            b_t = gath_pool.tile([P, B], xdt, tag="b")
                            nc.gpsimd.indirect_dma_start(
                                out=b_t[:],
                                out_offset=None,
                                in_=xt_d[:],
                                in_offset=bass.IndirectOffsetOnAxis(
                                    ap=idxb_t[:, gb : gb + 1], axis=0
                                ),
                            )

                        if mode == "fp16" and SPLITC == 2:
                            # half-batch tiles: transposes/copies of half 0
                            # overlap the u/v/p/s compute of half 1
                            HB = B // 2
                            s_halves = []
                            for h in range(2):
                                hsl = slice(h * HB, (h + 1) * HB)
                                uh = tmp_pool.tile([P, HB], f16, tag=f"u{h}")
                                nc.scalar.activation(
                                    uh[:], a_t[:, hsl],
                                    mybir.ActivationFunctionType.Identity,
                                    bias=KB, scale=KAB,
                                )
                                vh = tmp_pool.tile([P, HB], f16, tag=f"v{h}")
                                nc.vector.tensor_scalar(
                                    vh[:], a_t[:, hsl], KA, K0,
                                    op0=mybir.AluOpType.mult,
                                    op1=mybir.AluOpType.add,
                                )
                                ph = tmp_pool.tile([P, HB], f16, tag=f"p{h}")
                                nc.vector.tensor_tensor(
                                    out=ph[:], in0=uh[:], in1=b_t[:, hsl],
                                    op=mybir.AluOpType.mult,
                                )
                                sh = tmp_pool.tile([P, HB], f16, tag=f"s{h}")
                                nc.vector.tensor_tensor(
                                    out=sh[:], in0=ph[:], in1=vh[:],
                                    op=mybir.AluOpType.add,
                                )
                                s_halves.append(sh)
                        elif mode == "fp16":
                            # u = KAB*ta + KB   (ScalarE, fp16)
                            u_t = tmp_pool.tile([P, B], f16, tag="u")
                            nc.scalar.activation(
                                u_t[:], a_t[:],
                                mybir.ActivationFunctionType.Identity,
                                bias=KB, scale=KAB,
                            )
                            if GATHER_MULT:
                                # p = u * tb computed by the DMA: the b-gather
                                # multiplies into u's tile (CCE inline ALU)
                                nc.gpsimd.indirect_dma_start(
                                    out=u_t[:],
                                    out_offset=None,
                                    in_=xt_d[:],
                                    in_offset=bass.IndirectOffsetOnAxis(
                                        ap=idxb_t[:, gb : gb + 1], axis=0
                                    ),
                                    compute_op=mybir.AluOpType.mult,
                                )
                            # v = KA*ta + K0    (DVE tensor_scalar, fp16 4x)
                            v_t = tmp_pool.tile([P, B], f16, tag="v")
                            nc.vector.tensor_scalar(
                                v_t[:], a_t[:], KA, K0,
                                op0=mybir.AluOpType.mult,
                                op1=mybir.AluOpType.add,
                            )
                            # p = u*tb          (DVE fp16 2x, or folded into
                            # the b-gather via the SDMA CCE mult path)
                            if GATHER_MULT:
                                p_t = u_t
                            elif INPLACE_P:
                                p_t = u_t
                                nc.vector.tensor_tensor(
                                    out=p_t[:], in0=u_t[:], in1=b_t[:],
                                    op=mybir.AluOpType.mult,
                                )
                            else:
                                p_t = tmp_pool.tile([P, B], f16, tag="p")
                                nc.vector.tensor_tensor(
                                    out=p_t[:], in0=u_t[:], in1=b_t[:],
                                    op=mybir.AluOpType.mult,
                                )
                            if PSUM_ACC or REGMM:
                                s_t = None
                            else:
                                # s = p + v (DVE) so one transpose per block
                                if INPLACE_S:
                                    s_t = v_t
                                else:
                                    s_t = tmp_pool.tile([P, B], f16, tag="s")
                                s_eng = (
                                    nc.gpsimd if S_ENG == "gpsimd" else nc.vector
                                )
                                s_eng.tensor_tensor(
                                    out=s_t[:], in0=p_t[:], in1=v_t[:],
                                    op=mybir.AluOpType.add,
                                )
                        else:
                            # u = KAB*ta + KB   (ScalarE, f32)
                            u_t = tmp_pool.tile([P, B], f32, tag="u")
                            nc.scalar.activation(
                                u_t[:], a_t[:],
                                mybir.ActivationFunctionType.Identity,
                                bias=KB, scale=KAB,
                            )
                            # v = KA*ta + K0    (ScalarE, f32)
                            v_t = tmp_pool.tile([P, B], f32, tag="v")
                            nc.scalar.activation(
                                v_t[:], a_t[:],
                                mybir.ActivationFunctionType.Identity,
                                bias=K0, scale=KA,
                            )
                            # p = u*tb          (DVE, f32)
                            p_t = tmp_pool.tile([P, B], f32, tag="p")
                            nc.vector.tensor_tensor(
                                out=p_t[:], in0=u_t[:], in1=b_t[:],
                                op=mybir.AluOpType.mult,
                            )

                        for q in range(4):
                            if mode == "fp16" and REGMM:
                                ps = psum_pool.tile(
                                    [P, 1024], f32, space="PSUM", tag="ps"
                                )
                                for j in range(8):
                                    col = q * 1024 + j * P
                                    nc.tensor.matmul(
                                        out=ps[:, j * P : (j + 1) * P],
                                        lhsT=p_t[:, col : col + P],
                                        rhs=ident_p[:],
                                        start=True,
                                        stop=False,
                                    )
                                    nc.tensor.matmul(
                                        out=ps[:, j * P : (j + 1) * P],
                                        lhsT=v_t[:, col : col + P],
                                        rhs=ident_p[:],
                                        start=False,
                                        stop=True,
                                    )
                            elif mode == "fp16" and COPY2Q:
                                if q % 2 == 0:
                                    ps2 = psum_pool.tile(
                                        [P, 2048], f16, space="PSUM", tag="ps"
                                    )
                                ps = ps2[:, (q % 2) * 1024 : (q % 2) * 1024 + 1024]
                                for j in range(8):
                                    col = q * 1024 + j * P
                                    nc.tensor.matmul(
                                        out=ps[:, j * P : (j + 1) * P],
                                        lhsT=s_t[:, col : col + P],
                                        rhs=ident_p[:],
                                        is_transpose=True,
                                        start=True,
                                        stop=True,
                                    )
                                if q % 2 == 0:
                                    continue
                            elif mode == "fp16":
                                ps = psum_pool.tile(
                                    [P, 1024], f16, space="PSUM", tag="ps"
                                )
                                for j in range(8):
                                    col = q * 1024 + j * P
                                    if PSUM_ACC:
                                        nc.tensor.matmul(
                                            out=ps[:, j * P : (j + 1) * P],
                                            lhsT=p_t[:, col : col + P],
                                            rhs=ident_p[:],
                                            is_transpose=True,
                                            start=True,
                                            stop=False,
                                        )
                                        nc.tensor.matmul(
                                            out=ps[:, j * P : (j + 1) * P],
                                            lhsT=v_t[:, col : col + P],
                                            rhs=ident_p[:],
                                            is_transpose=True,
                                            start=False,
                                            stop=True,
                                        )
                                    else:
                                        if SPLITC == 2:
                                            hcol = col - (q // 2) * 2048
                                            lhs = s_halves[q // 2][
                                                :, hcol : hcol + P
                                            ]
                                        else:
                                            lhs = s_t[:, col : col + P]
                                        nc.tensor.matmul(
                                            out=ps[:, j * P : (j + 1) * P],
                                            lhsT=lhs,
                                            rhs=ident_p[:],
                                            is_transpose=True,
                                            start=True,
                                            stop=True,
                                        )
                            else:
                                ps = psum_pool.tile(
                                    [P, 1024], f32, space="PSUM", tag="ps"
                                )
                                for j in range(8):
                                    col = q * 1024 + j * P
                                    nc.tensor.matmul(
                                        out=ps[:, j * P : (j + 1) * P],
                                        lhsT=p_t[:, col : col + P],
                                        rhs=ident_p[:],
                                        is_transpose=True,
                                        start=True,
                                        stop=False,
                                    )
                                    nc.tensor.matmul(
                                        out=ps[:, j * P : (j + 1) * P],
                                        lhsT=v_t[:, col : col + P],
                                        rhs=ident_v[:],
                                        is_transpose=True,
                                        start=False,
                                        stop=True,
                                    )
                            qper = 4 // n_ot
                            if mode == "fp16" and COPY2Q:
                                # q is odd here: copy quarters q-1 and q at once
                                q0 = q - 1
                                assert qper >= 2, "COPY2Q needs OUT_MODE pair/one"
                                dst = qts[q0 // qper][
                                    :,
                                    (q0 % qper) * 8 : (q0 % qper) * 8 + 16,
                                    g4 * P : (g4 + 1) * P,
                                ]
                                src = ps2[:].rearrange("p (j g) -> p j g", j=16)
                            else:
                                dst = qts[q // qper][
                                    :,
                                    (q % qper) * 8 : (q % qper) * 8 + 8,
                                    g4 * P : (g4 + 1) * P,
                                ]
                                src = ps[:].rearrange("p (j g) -> p j g", j=8)
                            idx = (
                                gb * 2 + (q // 2)
                                if (mode == "fp16" and COPY2Q)
                                else gb * 4 + q
                            )
                            if mode != "fp16":
                                on_sce = idx % 3 == 0
                            elif COPY_PAT == "sce2of3":
                                on_sce = idx % 3 != 0
                            elif COPY_PAT == "sce3of4":
                                on_sce = idx % 4 != 0
                            elif COPY_PAT == "dve2of3":
                                on_sce = idx % 3 == 0
                            else:
                                on_sce = idx % 2 == 0
                            if on_sce:
                                nc.scalar.copy(dst, src)
                            else:
                                nc.vector.tensor_copy(dst, src)

                    if OUT_SCRAMBLE:
                        nc.sync.dma_start(
                            out=out_d[:][grp * P : (grp + 1) * P, :],
                            in_=qts[0][:].rearrange("p j g -> p (j g)"),
                        )
                    else:
                        gsl = slice(grp * GRP * P, (grp + 1) * GRP * P)
                        for pr in range(n_ot):
                            nc.sync.dma_start(
                                out=out_v[:, pr * jspan : (pr + 1) * jspan, gsl],
                                in_=qts[pr][:],
                            )
    nc.compile()
    return nc


def _get_program(reps=1, mode=MODE):
    key = (reps, mode)
    if key not in _PROGRAMS:
        _PROGRAMS[key] = _build_program(reps, mode)
    return _PROGRAMS[key]


def _host_prep(x, weights, idx_a, idx_b, mode=MODE):
    x = np.asarray(x, dtype=np.float32)
    if mode == "fp16":
        xt = np.ascontiguousarray((x.T.astype(np.float64) - 0.5).astype(np.float16))
        al = 1.0
    else:
        u = np.clip(np.rint((x.astype(np.float64) - 0.5) * 65536.0), -32768, 32767)
        xt = np.ascontiguousarray(u.astype(np.int16).T)
        al = 1.0 / 65536.0

    # truth table: T[i, j] = bit (3-j) of i
    tbl = ((np.arange(16)[:, None] >> (3 - np.arange(4))[None, :]) & 1).astype(
        np.float64
    )
    w = np.asarray(weights, dtype=np.float64)
    w = w - w.max(axis=-1, keepdims=True)
    e = np.exp(w)
    p = e / e.sum(axis=-1, keepdims=True)
    c = p @ tbl  # [O, 4]
    k0 = c[:, 0]
    ka = c[:, 2] - c[:, 0]
    kb = c[:, 1] - c[:, 0]
    kab = c[:, 0] - c[:, 1] - c[:, 2] + c[:, 3]
    K0 = k0 + ka / 2 + kb / 2 + kab / 4
    KA = al * (ka + kab / 2)
    KB = al * (kb + kab / 2)
    KAB = al * al * kab
    coef = np.stack([K0, KA, KB, KAB], axis=1).astype(np.float32)  # [O, 4]

    ia = np.asarray(idx_a, dtype=np.int32)
    ib = np.asarray(idx_b, dtype=np.int32)
    return xt, coef, ia, ib


def make_in_maps(x, weights, idx_a, idx_b, mode=MODE):
    xt, coef, ia, ib = _host_prep(x, weights, idx_a, idx_b, mode)
    in_maps = []
    for k in range(NCORES):
        osl = slice(k * OSH, (k + 1) * OSH)
        # swizzle: gate g (within shard) = gb*128 + p  ->  [p, gb]
        ia_k = np.ascontiguousarray(ia[osl].reshape(GBLOCKS, P).T)
        ib_k = np.ascontiguousarray(ib[osl].reshape(GBLOCKS, P).T)
        # coef: [GBLOCKS, P, 4] -> [P, GBLOCKS, 4] -> [P, GBLOCKS*4]
        coef_k = np.ascontiguousarray(
            coef[osl].reshape(GBLOCKS, P, 4).transpose(1, 0, 2).reshape(P, GBLOCKS * 4)
        )
        iab_k = np.ascontiguousarray(
            np.stack([ia_k, ib_k], axis=2).reshape(P, 2 * GBLOCKS)
        )
        in_maps.append(
            {"xt": xt, "idxa": ia_k, "idxb": ib_k, "idxab": iab_k, "coef": coef_k}
        )
    return in_maps


def device_out_to_full(core_out):
    """Per-core device 'out' array -> [B, OSH] f32 (handles both layouts)."""
    core_out = np.asarray(core_out)
    if not OUT_SCRAMBLE:
        return core_out.astype(np.float32)
    ngrp = GBLOCKS // GRP
    a = core_out.reshape(ngrp, P, 32, GRP * P)
    return (
        a.transpose(2, 1, 0, 3).reshape(B, OSH).astype(np.float32)
    )


def run_kernel(x, weights, idx_a, idx_b, trace=False, mode=MODE):
    """Returns (out, BassKernelResults)."""
    in_maps = make_in_maps(x, weights, idx_a, idx_b, mode)
    nc = _get_program(1, mode)
    try:
        res = run_bass_kernel_spmd(nc, in_maps, list(range(NCORES)), trace=trace)
    except Exception:
        # transient device/tunnel hiccups (e.g. NRT_EXEC_UNIT_UNRECOVERABLE)
        # have been observed once; one retry is cheap insurance.
        res = run_bass_kernel_spmd(nc, in_maps, list(range(NCORES)), trace=trace)
    out = np.concatenate(
        [device_out_to_full(res.results[k]["out"]) for k in range(NCORES)], axis=1
    )
    return out, res


def kernel(x, weights, idx_a, idx_b):
    # fp16 centered encoding degrades gracefully for any x range; the spec'd
    # fill is uniform [0, 1), for which max rel err measures 3.6e-3.
    out, _ = run_kernel(x, weights, idx_a, idx_b, trace=False)
    return out

